# revision 9
# baseline (speedup 1.0000x reference)
"""MultiHeadAttention (B=2, L=2048, DX=1024, H=16, DK=64) on 8 TRN2 NeuronCores.

Sharding: core c -> batch b = c//4, heads 4r..4r+3 where r = c%4.
Data-parallel over B, tensor-parallel over heads; O-projection partials are
ReduceScatter-summed over each 4-core group, LN done on the 512-row shard.

On-chip layout choices:
  - Projections produce Q^T, K^T ([dk, L], dk on partitions) and V ([L, dk],
    tokens on partitions) so that scores are computed transposed
    (S^T = K Q^T, [k, q]) and the exp'd score tiles feed the AV matmul
    directly as the moving operand -- no on-chip transposes anywhere.
  - V carries an extra ones-column so the AV matmul also produces softmax
    row-sums for free (psum row 64).
  - The faithful-to-reference mask quirk: head h uses pad_mask[h % 2]
    (mask tiled head-major over batch-major scores). Mask is applied
    multiplicatively on exp(S/8) with a 0/1 uint8 mask.
  - attn is returned transposed per head ([k, q]) and fixed up host-side.

kernel(**inputs) matches reference(): returns (out, attn).
"""

import sys

sys.path.insert(0, "/opt/trn_rl_repo")

import numpy as np

import concourse.bass as bass
import concourse.tile as tile
from concourse import bacc, mybir
from concourse.bass_utils import run_bass_kernel_spmd

B = 2
DX = 1024
H = 16
DK = 64
LN_EPS = 1e-5
N_CORES = 8
HPC = 4  # heads per core

USE_F32R = True  # fp32r matmuls: full PE speed at N>=256, ~tf32 precision

F32 = mybir.dt.float32
F32R = mybir.dt.float32r
U8 = mybir.dt.uint8


def _mm_dt(ap):
    return ap.bitcast(F32R) if USE_F32R else ap


def build(L=2048, n_cores=N_CORES):
    """Build the SPMD program (identical on all cores)."""
    KC = L // 128          # k-token chunks
    QC = L // 512          # q-token chunks of 512
    LR = L // 4            # rows per core after reduce-scatter
    DXC = DX // 128        # contraction chunks for projections

    nc = bacc.Bacc("TRN2", target_bir_lowering=False, debug=False,
                   num_devices=n_cores)

    ap_in = lambda name, shape, dt=F32: nc.dram_tensor(
        name, shape, dt, kind="ExternalInput").ap()
    mdt = F32R if USE_F32R else F32
    qT = ap_in("qT", [DX, L], mdt)
    kT = ap_in("kT", [DX, L], mdt)
    vT = ap_in("vT", [DX, L], mdt)
    maskT = ap_in("maskT", [2, L, L], U8)
    wq = ap_in("wq", [DX, 256], mdt)
    wk = ap_in("wk", [DX, 256], mdt)
    wv = ap_in("wv", [DX, 256], mdt)
    bq = ap_in("bq", [256])
    bk = ap_in("bk", [256])
    bv = ap_in("bv", [256])
    wo = ap_in("wo", [256, DX], mdt)
    bo = ap_in("bo", [DX])
    ln_g = ap_in("ln_g", [DX])
    ln_b = ap_in("ln_b", [DX])
    x_res = ap_in("x", [LR, DX])

    attnT = nc.dram_tensor("attnT", [HPC, L, L], F32,
                           kind="ExternalOutput").ap()
    out = nc.dram_tensor("out", [LR, DX], F32, kind="ExternalOutput").ap()

    groups = [[0, 1, 2, 3], [4, 5, 6, 7]] if n_cores == 8 else \
        [list(range(n_cores))]

    def bcast_row(dram_ap, n):
        """DRAM [n] -> AP reading it replicated across 128 partitions."""
        return bass.AP(tensor=dram_ap.tensor, offset=dram_ap.offset,
                       ap=[[0, 128], dram_ap.ap[0]])

    from contextlib import ExitStack
    with tile.TileContext(nc) as tc, ExitStack() as ctx:
        persist = ctx.enter_context(tc.tile_pool(name="persist", bufs=1))
        psum = ctx.enter_context(tc.tile_pool(name="psum", bufs=4,
                                              space="PSUM"))
        av_ps_pool = ctx.enter_context(tc.tile_pool(name="avps", bufs=2,
                                                    space="PSUM"))
        dram = ctx.enter_context(tc.tile_pool(name="dram", bufs=1,
                                              space="DRAM"))

        # ---- persistent sbuf ----
        wq_sb = persist.tile([128, DXC, 256], mdt)
        wk_sb = persist.tile([128, DXC, 256], mdt)
        wv_sb = persist.tile([128, DXC, 256], mdt)
        wo_sb = persist.tile([128, 2, DX], mdt)
        nc.sync.dma_start(out=wq_sb, in_=wq.rearrange("(c p) m -> p c m", p=128))
        nc.sync.dma_start(out=wk_sb, in_=wk.rearrange("(c p) m -> p c m", p=128))
        nc.sync.dma_start(out=wv_sb, in_=wv.rearrange("(c p) m -> p c m", p=128))
        nc.sync.dma_start(out=wo_sb, in_=wo.rearrange("(m p) n -> p m n", p=128))
        bq_sb = persist.tile([128, 2], F32)
        bk_sb = persist.tile([128, 2], F32)
        nc.sync.dma_start(out=bq_sb, in_=bq.rearrange("(m p) -> p m", p=128))
        nc.sync.dma_start(out=bk_sb, in_=bk.rearrange("(m p) -> p m", p=128))
        bv_rep = persist.tile([128, 256], F32)
        nc.sync.dma_start(out=bv_rep, in_=bcast_row(bv, 256))

        QT = persist.tile([128, 2, L], mdt)    # chunk m: heads (2m, 2m+1)
        KT = persist.tile([128, 2, L], mdt)
        Vb = persist.tile([128, KC, HPC * 65], mdt)  # per kc: [V_h | 1] x 4
        AVT = persist.tile([128, 2, L], mdt)
        eps_sb = persist.tile([128, 1], F32)
        nc.vector.memset(eps_sb, LN_EPS)
        ones1 = persist.tile([128, 1], F32)
        nc.vector.memset(ones1, 1.0)
        for h in range(HPC):
            nc.vector.tensor_copy(
                out=Vb[:, :, 65 * h + 64: 65 * h + 65],
                in_=bass.AP(tensor=ones1.tensor, offset=ones1.offset,
                            ap=[ones1.ap[0], [0, KC], ones1.ap[1]]))

        # ---- P1: projections ----
        with tc.tile_pool(name="stream", bufs=2) as stream:
            # Q^T and K^T: [dk, L] with dk on partitions.
            for src, w_sb, b_sb, dst in ((qT, wq_sb, bq_sb, QT),
                                         (kT, wk_sb, bk_sb, KT)):
                for j in range(QC):
                    qs = 512 * j
                    xt = stream.tile([128, DXC, 512], mdt, tag="xt")
                    nc.sync.dma_start(
                        out=xt,
                        in_=src.rearrange("(c p) n -> p c n", p=128)[:, :, qs:qs + 512])
                    for m in range(2):
                        ps = psum.tile([128, 512], F32, tag="ps")
                        for c in range(DXC):
                            nc.tensor.matmul(
                                ps,
                                w_sb[:, c, 128 * m:128 * m + 128],
                                xt[:, c, :],
                                start=(c == 0), stop=(c == DXC - 1))
                        nc.vector.tensor_scalar_add(
                            out=dst[:, m, qs:qs + 512], in0=ps,
                            scalar1=b_sb[:, m:m + 1])
            # V: [L, dk] with tokens on partitions (+ bias, split into blocks)
            for kc in range(KC):
                vt = stream.tile([128, DXC, 128], mdt, tag="xt")
                nc.sync.dma_start(
                    out=vt,
                    in_=vT.rearrange("(c p) n -> p c n", p=128)[:, :, 128 * kc:128 * kc + 128])
                ps_full = psum.tile([128, 512], F32, tag="ps")
                ps = ps_full[:, :256]
                for c in range(DXC):
                    nc.tensor.matmul(
                        ps, vt[:, c, :], wv_sb[:, c, :],
                        start=(c == 0), stop=(c == DXC - 1))
                for h in range(HPC):
                    nc.vector.tensor_add(
                        out=Vb[:, kc, 65 * h:65 * h + 64],
                        in0=ps[:, 64 * h:64 * h + 64],
                        in1=bv_rep[:, 64 * h:64 * h + 64])

        # ---- P2: attention ----
        with tc.tile_pool(name="epool", bufs=2) as epool, \
             tc.tile_pool(name="mpool", bufs=2) as mpool, \
             tc.tile_pool(name="rpool", bufs=2) as rpool, \
             tc.tile_pool(name="rdram", bufs=3, space="DRAM") as rdram:
            for j in range(QC):
                qs = 512 * j
                for par in range(2):
                    mask_t = mpool.tile([128, KC, 512], U8, tag="m")
                    nc.sync.dma_start(
                        out=mask_t,
                        in_=maskT[par].rearrange("(kc p) q -> p kc q", p=128)[:, :, qs:qs + 512])
                    for h in (par, par + 2):
                        hp = 64 * (h % 2)
                        E_t = epool.tile([128, KC, 512], F32, tag="e")
                        for kc in range(KC):
                            s_ps = psum.tile([128, 512], F32, tag="ps")
                            nc.tensor.matmul(
                                s_ps,
                                KT[hp:hp + 64, h // 2, 128 * kc:128 * kc + 128],
                                QT[hp:hp + 64, h // 2, qs:qs + 512],
                                start=True, stop=True)
                            nc.scalar.activation(
                                out=_mm_dt(E_t[:, kc, :]), in_=s_ps,
                                func=mybir.ActivationFunctionType.Exp,
                                scale=0.125)
                        nc.vector.tensor_mul(out=_mm_dt(E_t), in0=E_t, in1=mask_t)
                        av_ps = av_ps_pool.tile([128, 512], F32, tag="av")
                        for kc in range(KC):
                            nc.tensor.matmul(
                                av_ps[0:65, :],
                                Vb[:, kc, 65 * h:65 * h + 65],
                                _mm_dt(E_t[:, kc, :]),
                                start=(kc == 0), stop=(kc == KC - 1))
                        rs_t = rpool.tile([128, 512], F32, tag="rs")
                        rep_t = rpool.tile([128, 512], F32, tag="rep")
                        nc.vector.reciprocal(out=rs_t[64:65, :],
                                             in_=av_ps[64:65, :])
                        # exact partition-replicate via DRAM bounce (SBUF APs
                        # forbid step-0 partition reads; DRAM APs allow them)
                        rrow = rdram.tile([512], F32, tag="rrow")
                        nc.sync.dma_start(out=rrow, in_=rs_t[64:65, :])
                        nc.sync.dma_start(
                            out=rep_t,
                            in_=bass.AP(tensor=rrow.tensor, offset=rrow.offset,
                                        ap=[[0, 128], rrow.ap[-1]]))
                        rep_b = bass.AP(tensor=rep_t.tensor, offset=rep_t.offset,
                                        ap=[rep_t.ap[0], [0, KC], rep_t.ap[1]])
                        nc.vector.tensor_mul(out=_mm_dt(E_t), in0=E_t, in1=rep_b)
                        # 64-channel DVE op: reads parts 0-63, may write the
                        # upper quadrant pair (parity-1 heads) -- legal per
                        # the bank->quadrant routing table.
                        nc.vector.tensor_mul(
                            out=AVT[hp:hp + 64, h // 2, qs:qs + 512],
                            in0=av_ps[0:64, :], in1=rep_t[0:64, :])
                        nc.sync.dma_start(
                            out=attnT[h].rearrange("(kc p) q -> p kc q", p=128)[:, :, qs:qs + 512],
                            in_=E_t)

        # ---- P3: output projection, reduce-scatter, layernorm ----
        rs_in = dram.tile([L, DX], F32)
        rs_out = dram.tile([LR, DX], F32)
        with tc.tile_pool(name="op_ev", bufs=3) as op_ev:
            for qm in range(L // 128):
                for dc in range(DX // 512):
                    ps = psum.tile([128, 512], F32, tag="ps")
                    for m in range(2):
                        nc.tensor.matmul(
                            ps,
                            AVT[:, m, 128 * qm:128 * qm + 128],
                            wo_sb[:, m, 512 * dc:512 * dc + 512],
                            start=(m == 0), stop=(m == 1))
                    ev = op_ev.tile([128, 512], F32, tag="ev")
                    nc.scalar.copy(out=ev, in_=ps)
                    nc.sync.dma_start(
                        out=rs_in[128 * qm:128 * qm + 128,
                                  512 * dc:512 * dc + 512],
                        in_=ev)
        nc.gpsimd.collective_compute(
            "ReduceScatter", mybir.AluOpType.add, replica_groups=groups,
            ins=[rs_in.opt()], outs=[rs_out.opt()])

        with tc.tile_pool(name="lnp", bufs=2) as lnp:
            bo_rep = lnp.tile([128, DX], F32, tag="bo")
            g_rep = lnp.tile([128, DX], F32, tag="g")
            b_rep = lnp.tile([128, DX], F32, tag="b")
            nc.sync.dma_start(out=bo_rep, in_=bcast_row(bo, DX))
            nc.sync.dma_start(out=g_rep, in_=bcast_row(ln_g, DX))
            nc.sync.dma_start(out=b_rep, in_=bcast_row(ln_b, DX))
            fmax = int(nc.vector.BN_STATS_FMAX)
            import math
            sub = math.gcd(fmax, DX)
            nsub = DX // sub
            for t in range(LR // 128):
                y = lnp.tile([128, DX], F32, tag="y")
                xt = lnp.tile([128, DX], F32, tag="x")
                nc.sync.dma_start(out=y, in_=rs_out[128 * t:128 * t + 128, :])
                nc.sync.dma_start(out=xt, in_=x_res[128 * t:128 * t + 128, :])
                nc.vector.tensor_add(out=y, in0=y, in1=xt)
                nc.vector.tensor_add(out=y, in0=y, in1=bo_rep)
                stats = lnp.tile([128, nsub, 6], F32, tag="st")
                yv = y.rearrange("p (s d) -> p s d", s=nsub)
                for s in range(nsub):
                    nc.vector.bn_stats(out=stats[:, s, :], in_=yv[:, s, :])
                mv = lnp.tile([128, 2], F32, tag="mv")
                nc.vector.bn_aggr(out=mv, in_=stats)
                std = lnp.tile([128, 1], F32, tag="sd")
                nc.scalar.activation(out=std, in_=mv[:, 1:2],
                                     func=mybir.ActivationFunctionType.Sqrt,
                                     bias=eps_sb, scale=1.0)
                nc.vector.reciprocal(out=std, in_=std)
                nc.vector.tensor_scalar(out=y, in0=y, scalar1=mv[:, 0:1],
                                        scalar2=std,
                                        op0=mybir.AluOpType.subtract,
                                        op1=mybir.AluOpType.mult)
                nc.vector.tensor_mul(out=y, in0=y, in1=g_rep)
                nc.vector.tensor_add(out=y, in0=y, in1=b_rep)
                nc.sync.dma_start(out=out[128 * t:128 * t + 128, :], in_=y)

    nc.compile()
    return nc


_CACHE = {}


def _get_nc(L=2048, n_cores=N_CORES):
    key = (L, n_cores, USE_F32R)
    if key not in _CACHE:
        _CACHE[key] = build(L, n_cores)
    return _CACHE[key]


def make_in_maps(q, k, v, pad_mask, wq_w, wq_b, wk_w, wk_b, wv_w, wv_b,
                 wo_w, wo_b, ln_g, ln_b):
    L = q.shape[1]
    LR = L // 4
    keepT = np.ascontiguousarray(
        (~pad_mask).astype(np.uint8).transpose(0, 2, 1))  # [2, Lk, Lq]
    qT = [np.ascontiguousarray(q[b].T) for b in range(B)]
    kTa = [np.ascontiguousarray(k[b].T) for b in range(B)]
    vTa = [np.ascontiguousarray(v[b].T) for b in range(B)]
    in_maps = []
    for c in range(N_CORES):
        b, r = c // 4, c % 4
        sl = slice(256 * r, 256 * r + 256)
        in_maps.append({
            "qT": qT[b], "kT": kTa[b], "vT": vTa[b], "maskT": keepT,
            "wq": np.ascontiguousarray(wq_w[:, sl]),
            "wk": np.ascontiguousarray(wk_w[:, sl]),
            "wv": np.ascontiguousarray(wv_w[:, sl]),
            "bq": np.ascontiguousarray(wq_b[sl]),
            "bk": np.ascontiguousarray(wk_b[sl]),
            "bv": np.ascontiguousarray(wv_b[sl]),
            "wo": np.ascontiguousarray(wo_w[sl, :]),
            "bo": wo_b, "ln_g": ln_g, "ln_b": ln_b,
            "x": np.ascontiguousarray(q[b, LR * r:LR * r + LR]),
        })
    return in_maps


def kernel(q, k, v, pad_mask, wq_w, wq_b, wk_w, wk_b, wv_w, wv_b,
           wo_w, wo_b, ln_g, ln_b, _trace=False, _trace_kwargs=None):
    args = [np.asarray(a) for a in (q, k, v, pad_mask, wq_w, wq_b, wk_w, wk_b,
                                    wv_w, wv_b, wo_w, wo_b, ln_g, ln_b)]
    q, k, v, pad_mask = args[0], args[1], args[2], args[3]
    L = q.shape[1]
    nc = _get_nc(L)
    in_maps = make_in_maps(*args)
    res = run_bass_kernel_spmd(nc, in_maps, list(range(N_CORES)),
                               trace=_trace, **(_trace_kwargs or {}))
    out = np.empty((B, L, DX), np.float32)
    attn = np.empty((B, H, L, L), np.float32)
    LR = L // 4
    for c in range(N_CORES):
        b, r = c // 4, c % 4
        out[b, LR * r:LR * r + LR] = res.results[c]["out"]
        attn[b, 4 * r:4 * r + 4] = res.results[c]["attnT"].transpose(0, 2, 1)
    kernel.last_results = res
    return out, attn
